# revision 1
# baseline (speedup 1.0000x reference)
"""Block-diagonal MLP kernel for Trainium2 (8 NeuronCores, data-parallel).

Computes out = blockdiag_matmul(x, weights) + bias where
  x: [4, 2048, 4096] f32, weights: [32, 128, 128] f32, bias: [4096] f32.

Strategy: shard the 8192 flattened batch rows across 8 cores (1024 rows
each), replicate weights/bias.  Per core, process 8 row-tiles of
[128, 4096]:
  - DMA x tile in (natural layout, max-size contiguous transfers)
  - PE transpose-mode matmuls turn each [128,128] feature block into
    feature-major layout (the matmul contraction dim must be the
    partition dim), 4 blocks per PSUM bank
  - ACT evacuates the transposed chunk to SBUF
  - fp32 matmuls against the SBUF-resident weights, 4 blocks per bank
  - DVE evacuates with the bias add fused
  - DMA out tile (stores alternate between the two HWDGE rings)
The per-group work is software-pipelined (transposes emitted two groups
ahead of the consuming matmuls) so the PE stream stays dense.  Exactly
matches the fp32 jax reference bit-for-bit (same fp32 matmul path).
"""
import numpy as np
from contextlib import ExitStack

import concourse.mybir as mybir
import concourse.tile as tile
from concourse import bacc
from concourse.bass_utils import run_bass_kernel_spmd
from concourse.masks import make_identity

F32 = mybir.dt.float32

SIZE = 4096
NB = 32          # number of diagonal blocks
BLK = 128        # block size
N_CORES = 8
B_FULL = 4 * 2048            # 8192 flattened rows
B_CORE = B_FULL // N_CORES   # 1024 rows per core
ROW_TILES = B_CORE // 128    # 8 tiles of 128 rows
GROUPS = SIZE // 512         # 8 groups of 4 blocks (512 cols) per row-tile

_NC_CACHE = {}


def _build_nc():
    nc = bacc.Bacc()
    x_d = nc.declare_dram_parameter("x", [B_CORE, SIZE], F32, isOutput=False)
    # weights pre-transposed on host to [d, k*128+e]; bias pre-replicated
    # to [128, SIZE] — both load as single fully-contiguous transfers.
    w_d = nc.declare_dram_parameter("weights", [BLK, NB * BLK], F32, isOutput=False)
    b_d = nc.declare_dram_parameter("bias", [128, SIZE], F32, isOutput=False)
    o_d = nc.declare_dram_parameter("out", [B_CORE, SIZE], F32, isOutput=True)

    with tile.TileContext(nc) as tc, ExitStack() as ctx:
        consts = ctx.enter_context(tc.tile_pool(name="consts", bufs=1))
        x_pool = ctx.enter_context(tc.tile_pool(name="x", bufs=3))
        xt_pool = ctx.enter_context(tc.tile_pool(name="xt", bufs=4))
        out_pool = ctx.enter_context(tc.tile_pool(name="out", bufs=3))
        tp_pool = ctx.enter_context(tc.tile_pool(name="tp", bufs=3, space="PSUM"))
        mp_pool = ctx.enter_context(tc.tile_pool(name="mp", bufs=4, space="PSUM"))

        # Identity first (gpsimd, cheap) — needed by the very first transpose.
        ident = consts.tile([BLK, BLK], F32)
        make_identity(nc, ident)
        # Weights (host pre-transposed to d-major) then bias (host
        # pre-replicated), each one fully-contiguous 2 MiB transfer on the
        # ACT HWDGE ring.
        w_sb = consts.tile([BLK, NB * BLK], F32)
        bias_sb = consts.tile([128, SIZE], F32)
        nc.scalar.dma_start(out=w_sb, in_=w_d[:, :])
        nc.scalar.dma_start(out=bias_sb, in_=b_d[:, :])

        for t in range(ROW_TILES):
            x_tile = x_pool.tile([128, SIZE], F32)
            # Tile 0 loads a small first chunk so the first transposes start
            # sooner; steady-state tiles load as one max-size transfer.
            if t == 0:
                nc.sync.dma_start(
                    out=x_tile[:, 0:512], in_=x_d[0:128, 0:512]
                )
                nc.sync.dma_start(
                    out=x_tile[:, 512:SIZE], in_=x_d[0:128, 512:SIZE]
                )
            else:
                nc.sync.dma_start(out=x_tile, in_=x_d[t * 128:(t + 1) * 128, :])
            out_tile = out_pool.tile([128, SIZE], F32)
            # Software-pipelined by one group: transposes for group g+1 are
            # emitted before group g's matmuls, so the PE keeps busy when a
            # matmul is briefly blocked on the xT copy or weights.
            def emit_transposes(g):
                tp = tp_pool.tile([128, 512], F32)
                for j in range(4):
                    k = 4 * g + j
                    nc.tensor.matmul(
                        tp[:, j * 128:(j + 1) * 128],
                        x_tile[:, k * 128:(k + 1) * 128],
                        ident,
                        is_transpose=True,
                        start=(j == 0),
                        stop=(j == 3),
                    )
                xt = xt_pool.tile([128, 512], F32)
                nc.scalar.copy(xt, tp)
                return xt
            xt_q = [emit_transposes(0), emit_transposes(1)]
            for g in range(GROUPS):
                xt = xt_q.pop(0)
                if g + 2 < GROUPS:
                    xt_q.append(emit_transposes(g + 2))
                # 4 block matmuls into one PSUM bank: out chunk
                mp = mp_pool.tile([128, 512], F32)
                for j in range(4):
                    k = 4 * g + j
                    nc.tensor.matmul(
                        mp[:, j * 128:(j + 1) * 128],
                        xt[:, j * 128:(j + 1) * 128],
                        w_sb[:, k * 128:(k + 1) * 128],
                        start=(j == 0),
                        stop=(j == 3),
                    )
                # bias add fused into PSUM evacuation
                out_slice = out_tile[:, g * 512:(g + 1) * 512]
                bias_slice = bias_sb[:, g * 512:(g + 1) * 512]
                nc.vector.tensor_add(out_slice, mp, bias_slice)
            # Stores alternate between the two HWDGE rings so the final
            # stores don't serialize behind each other; the last tile goes
            # out in quarters so the kernel tail only waits on 256 KiB.
            rows = slice(t * 128, (t + 1) * 128)
            if t == ROW_TILES - 1:
                for q in range(4):
                    eng = nc.scalar if q % 2 == 0 else nc.sync
                    cols = slice(q * 1024, (q + 1) * 1024)
                    eng.dma_start(out=o_d[rows, cols], in_=out_tile[:, cols])
            else:
                eng = nc.scalar if t % 2 == 0 else nc.sync
                eng.dma_start(out=o_d[rows, :], in_=out_tile)

    nc.compile()
    return nc


def _get_nc():
    if "nc" not in _NC_CACHE:
        _NC_CACHE["nc"] = _build_nc()
    return _NC_CACHE["nc"]


def _run(inputs, trace=False):
    x = np.asarray(inputs["x"], dtype=np.float32)
    weights = np.asarray(inputs["weights"], dtype=np.float32)
    bias = np.asarray(inputs["bias"], dtype=np.float32)
    orig_shape = x.shape
    xf = np.ascontiguousarray(x.reshape(B_FULL, SIZE))
    # Host-side layout for the small constants: weights d-major so the
    # SBUF tile loads contiguously, bias replicated across partitions.
    w_t = np.ascontiguousarray(
        weights.transpose(1, 0, 2).reshape(BLK, NB * BLK)
    )
    bias_rep = np.ascontiguousarray(np.broadcast_to(bias[None, :], (128, SIZE)))

    nc = _get_nc()
    in_maps = [
        {
            "x": xf[i * B_CORE:(i + 1) * B_CORE],
            "weights": w_t,
            "bias": bias_rep,
        }
        for i in range(N_CORES)
    ]
    res = run_bass_kernel_spmd(
        nc, in_maps, core_ids=list(range(N_CORES)), trace=trace
    )
    out = np.concatenate([res.results[i]["out"] for i in range(N_CORES)], axis=0)
    return out.reshape(orig_shape), res


def kernel(**inputs):
    out, _ = _run(inputs, trace=False)
    return out



# revision 2
# speedup vs baseline: 1.0175x; 1.0175x over previous
"""Block-diagonal MLP kernel for Trainium2 (8 NeuronCores, expert-sharded).

Computes out = blockdiag_matmul(x, weights) + bias where
  x: [4, 2048, 4096] f32, weights: [32, 128, 128] f32, bias: [4096] f32.

Strategy: shard the 32 diagonal blocks across 8 cores (4 blocks = 512
feature columns each); every core sees all 8192 flattened rows of its
512-column slice.  Per-core DMA is then 16.78 MB in + 16.78 MB out +
~0.4 MB consts -- right at the ~358 GB/s per-core HBM roofline.

Per core, 64 row-tiles of [128, 512]:
  - x tile loads on the ACT HWDGE ring (ACT also evacuates transposes,
    so its ring must carry the *pre*-matmul traffic: a store here would
    FIFO-couple the evac stream to the post-matmul add and serialize
    the pipeline)
  - PE transpose-mode matmuls (fp32) put the contraction dim on
    partitions, 4 blocks per PSUM bank
  - ACT evacuates the transposed chunk to SBUF, casting fp32->bf16
    (free cast; bf16 halves the real matmul cost on the PE)
  - bf16 matmuls against SBUF-resident bf16 weights (host-cast)
  - DVE evacuates with the bias add fused (fp32 PSUM + fp32 bias)
  - out tile stores on the Sync HWDGE ring (pure post-matmul traffic)
Loads are dispatched 5 tiles ahead and transposes run 2 tiles ahead of
the consuming matmuls, so both DMA rings and the PE stream stay dense.

bf16 is only used for the matmul operands (accumulation stays fp32 in
PSUM); max rel err vs the fp32 reference is ~2e-3, well inside the
2e-2 gate.
"""
import numpy as np
import ml_dtypes
from contextlib import ExitStack

import concourse.mybir as mybir
import concourse.tile as tile
from concourse import bacc
from concourse.bass_utils import run_bass_kernel_spmd
from concourse.masks import make_identity

F32 = mybir.dt.float32
BF16 = mybir.dt.bfloat16

SIZE = 4096
NB = 32            # number of diagonal blocks
BLK = 128          # block size
N_CORES = 8
NB_CORE = NB // N_CORES        # 4 blocks per core
C_CORE = NB_CORE * BLK         # 512 feature columns per core
B_FULL = 4 * 2048              # 8192 flattened rows (all on every core)
ROW_TILES = B_FULL // 128      # 64 tiles of 128 rows

_NC_CACHE = {}


def _build_nc():
    nc = bacc.Bacc()
    x_d = nc.declare_dram_parameter("x", [B_FULL, C_CORE], F32, isOutput=False)
    # weights pre-transposed AND pre-cast on host to bf16 [d, j*128+e];
    # bias pre-replicated to [128, C_CORE] f32 (256 KiB, cheap).
    w_d = nc.declare_dram_parameter("weights", [BLK, C_CORE], BF16, isOutput=False)
    b_d = nc.declare_dram_parameter("bias", [128, C_CORE], F32, isOutput=False)
    o_d = nc.declare_dram_parameter("out", [B_FULL, C_CORE], F32, isOutput=True)

    with tile.TileContext(nc) as tc, ExitStack() as ctx:
        consts = ctx.enter_context(tc.tile_pool(name="consts", bufs=1))
        x_pool = ctx.enter_context(tc.tile_pool(name="x", bufs=8))
        xt_pool = ctx.enter_context(tc.tile_pool(name="xt", bufs=3))
        out_pool = ctx.enter_context(tc.tile_pool(name="out", bufs=4))
        tp_pool = ctx.enter_context(tc.tile_pool(name="tp", bufs=3, space="PSUM"))
        mp_pool = ctx.enter_context(tc.tile_pool(name="mp", bufs=3, space="PSUM"))

        # Identity first (gpsimd, cheap) -- needed by the very first transpose.
        ident = consts.tile([BLK, BLK], F32)
        make_identity(nc, ident)

        w_sb = consts.tile([BLK, C_CORE], BF16)
        bias_sb = consts.tile([128, C_CORE], F32)

        x_tiles = [None] * ROW_TILES

        def emit_load(t):
            xt_ = x_pool.tile([128, C_CORE], F32)
            if t == 0:
                # split so the very first transposes start half a transfer sooner
                nc.scalar.dma_start(out=xt_[:, 0:256], in_=x_d[0:128, 0:256])
                nc.scalar.dma_start(out=xt_[:, 256:512], in_=x_d[0:128, 256:512])
            else:
                nc.scalar.dma_start(out=xt_, in_=x_d[t * 128:(t + 1) * 128, :])
            x_tiles[t] = xt_

        # Head: x0/x1 first so the PE can start, then the consts (needed
        # ~2 us in), then more prefetch.
        emit_load(0)
        emit_load(1)
        nc.scalar.dma_start(out=w_sb, in_=w_d[:, :])
        nc.scalar.dma_start(out=bias_sb, in_=b_d[:, :])
        for t in range(2, 5):
            emit_load(t)

        def emit_transposes(t):
            tp = tp_pool.tile([128, C_CORE], F32)
            xtile = x_tiles[t]
            for j in range(NB_CORE):
                nc.tensor.matmul(
                    tp[:, j * 128:(j + 1) * 128],
                    xtile[:, j * 128:(j + 1) * 128],
                    ident,
                    is_transpose=True,
                    start=(j == 0),
                    stop=(j == NB_CORE - 1),
                )
            xt = xt_pool.tile([128, C_CORE], BF16)
            nc.scalar.copy(xt, tp)   # PSUM f32 -> SBUF bf16
            return xt

        xt_q = [emit_transposes(0), emit_transposes(1)]
        for t in range(ROW_TILES):
            if t + 5 < ROW_TILES:
                emit_load(t + 5)
            xt = xt_q.pop(0)
            if t + 2 < ROW_TILES:
                xt_q.append(emit_transposes(t + 2))
            mp = mp_pool.tile([128, C_CORE], F32)
            for j in range(NB_CORE):
                nc.tensor.matmul(
                    mp[:, j * 128:(j + 1) * 128],
                    xt[:, j * 128:(j + 1) * 128],
                    w_sb[:, j * 128:(j + 1) * 128],
                    start=(j == 0),
                    stop=(j == NB_CORE - 1),
                )
            out_tile = out_pool.tile([128, C_CORE], F32)
            nc.vector.tensor_add(out_tile, mp, bias_sb)
            rows = slice(t * 128, (t + 1) * 128)
            if t == ROW_TILES - 1:
                # ACT ring is drained of loads by now; split the final
                # store across both rings so the kernel tail halves.
                nc.sync.dma_start(out=o_d[rows, 0:256], in_=out_tile[:, 0:256])
                nc.scalar.dma_start(out=o_d[rows, 256:512], in_=out_tile[:, 256:512])
            else:
                nc.sync.dma_start(out=o_d[rows, :], in_=out_tile)

    nc.compile()
    return nc


def _get_nc():
    if "nc" not in _NC_CACHE:
        _NC_CACHE["nc"] = _build_nc()
    return _NC_CACHE["nc"]


def _run(inputs, trace=False):
    x = np.asarray(inputs["x"], dtype=np.float32)
    weights = np.asarray(inputs["weights"], dtype=np.float32)
    bias = np.asarray(inputs["bias"], dtype=np.float32)
    orig_shape = x.shape
    xf = x.reshape(B_FULL, SIZE)

    nc = _get_nc()
    in_maps = []
    for i in range(N_CORES):
        cols = slice(i * C_CORE, (i + 1) * C_CORE)
        # weights d-major per core: [d, j*128+e] = W[4i+j, d, e], cast bf16
        w_t = np.ascontiguousarray(
            weights[i * NB_CORE:(i + 1) * NB_CORE].transpose(1, 0, 2).reshape(BLK, C_CORE)
        ).astype(ml_dtypes.bfloat16)
        b_rep = np.ascontiguousarray(
            np.broadcast_to(bias[cols][None, :], (128, C_CORE))
        )
        in_maps.append(
            {
                "x": np.ascontiguousarray(xf[:, cols]),
                "weights": w_t,
                "bias": b_rep,
            }
        )
    res = run_bass_kernel_spmd(
        nc, in_maps, core_ids=list(range(N_CORES)), trace=trace
    )
    out = np.concatenate(
        [res.results[i]["out"] for i in range(N_CORES)], axis=1
    )
    return np.ascontiguousarray(out).reshape(orig_shape), res


def kernel(**inputs):
    out, _ = _run(inputs, trace=False)
    return out


# revision 3
# speedup vs baseline: 1.0873x; 1.0686x over previous
"""Block-diagonal MLP kernel for Trainium2 (8 NeuronCores, expert-sharded).

Computes out = blockdiag_matmul(x, weights) + bias where
  x: [4, 2048, 4096] f32, weights: [32, 128, 128] f32, bias: [4096] f32.

Strategy: shard the 32 diagonal blocks across 8 cores (4 blocks = 512
feature columns each); every core sees all 8192 flattened rows of its
512-column slice.  Per-core DMA is then 16.78 MB in + 16.78 MB out +
0.4 MB consts -- right at the ~360 GB/s per-core HBM roofline.

The host packs each core's x shard as [128, 32768] (partition p holds
the rows congruent to p mod 128, 64 row-groups side by side), so every
DMA moves 16 KiB-contiguous per-partition lines (2 KiB descriptors
measure ~65% slower per ring).  The whole shard fits in SBUF
(131 KiB/partition), so all loads are dispatched up front on the ACT
ring and stream in ahead of compute with no feedback coupling; stores
stream out on the Sync ring as [128, 4096] tiles.

Per 512-column group: PE transpose-mode matmuls (fp32) put the
contraction dim on partitions; ACT evacuates the transpose to SBUF
casting fp32->bf16 (free cast -- bf16 halves the real matmul cost);
bf16 matmuls against the SBUF-resident bf16 weights (host-cast, the
same 4 blocks for all 64 groups); DVE evacuates with the bias add
fused.  Transposes run two groups ahead of the consuming matmuls.
The final two tiles store in 512-column pieces alternating across both
rings so the kernel tail only waits on 256 KiB.

bf16 is only used for matmul operands (accumulation stays fp32 in
PSUM); max rel err vs the fp32 reference is ~2e-3 (gate: 2e-2).
"""
import numpy as np
import ml_dtypes
from contextlib import ExitStack

import concourse.mybir as mybir
import concourse.tile as tile
from concourse import bacc
from concourse.bass_utils import run_bass_kernel_spmd
from concourse.masks import make_identity

F32 = mybir.dt.float32
BF16 = mybir.dt.bfloat16

SIZE = 4096
NB = 32            # number of diagonal blocks
BLK = 128          # block size
N_CORES = 8
NB_CORE = NB // N_CORES        # 4 blocks per core
C_CORE = NB_CORE * BLK         # 512 feature columns per core
B_FULL = 4 * 2048              # 8192 flattened rows (all on every core)
GROUPS = B_FULL // 128         # 64 row-groups of [128, 512]
XP_COLS = GROUPS * C_CORE      # 32768 packed columns
G_PER_OUT = 8                  # groups per [128, 4096] out tile
OUT_TILES = GROUPS // G_PER_OUT

_NC_CACHE = {}


def _build_nc():
    nc = bacc.Bacc()
    x_d = nc.declare_dram_parameter("x", [128, XP_COLS], F32, isOutput=False)
    w_d = nc.declare_dram_parameter("weights", [BLK, C_CORE], BF16, isOutput=False)
    b_d = nc.declare_dram_parameter("bias", [128, C_CORE], F32, isOutput=False)
    o_d = nc.declare_dram_parameter("out", [128, XP_COLS], F32, isOutput=True)

    with tile.TileContext(nc) as tc, ExitStack() as ctx:
        consts = ctx.enter_context(tc.tile_pool(name="consts", bufs=1))
        xt_pool = ctx.enter_context(tc.tile_pool(name="xt", bufs=3))
        out_pool = ctx.enter_context(tc.tile_pool(name="out", bufs=2))
        tp_pool = ctx.enter_context(tc.tile_pool(name="tp", bufs=3, space="PSUM"))
        mp_pool = ctx.enter_context(tc.tile_pool(name="mp", bufs=3, space="PSUM"))

        # Identity first (gpsimd, cheap) -- needed by the very first transpose.
        ident = consts.tile([BLK, BLK], F32)
        make_identity(nc, ident)

        w_sb = consts.tile([BLK, C_CORE], BF16)
        bias_sb = consts.tile([128, C_CORE], F32)
        xp = consts.tile([128, XP_COLS], F32)   # whole x shard, SBUF-resident

        # All loads dispatched up front on the ACT ring: a small first
        # chunk so transposes start ~1 us in, the consts (needed ~2 us
        # in), then max-size streaming chunks.
        nc.scalar.dma_start(out=xp[:, 0:512], in_=x_d[:, 0:512])
        nc.scalar.dma_start(out=w_sb, in_=w_d[:, :])
        nc.scalar.dma_start(out=bias_sb, in_=b_d[:, :])
        nc.scalar.dma_start(out=xp[:, 512:2048], in_=x_d[:, 512:2048])
        nc.scalar.dma_start(out=xp[:, 2048:4096], in_=x_d[:, 2048:4096])
        for c in range(4096, XP_COLS, 4096):
            nc.scalar.dma_start(out=xp[:, c:c + 4096], in_=x_d[:, c:c + 4096])

        def emit_transposes(g):
            tp = tp_pool.tile([128, C_CORE], F32)
            for j in range(NB_CORE):
                nc.tensor.matmul(
                    tp[:, j * 128:(j + 1) * 128],
                    xp[:, g * C_CORE + j * 128:g * C_CORE + (j + 1) * 128],
                    ident,
                    is_transpose=True,
                    start=(j == 0),
                    stop=(j == NB_CORE - 1),
                )
            xt = xt_pool.tile([128, C_CORE], BF16)
            nc.scalar.copy(xt, tp)   # PSUM f32 -> SBUF bf16
            return xt

        xt_q = [emit_transposes(0), emit_transposes(1)]
        out_tile = None
        for g in range(GROUPS):
            if g % G_PER_OUT == 0:
                out_tile = out_pool.tile([128, G_PER_OUT * C_CORE], F32)
            xt = xt_q.pop(0)
            if g + 2 < GROUPS:
                xt_q.append(emit_transposes(g + 2))
            mp = mp_pool.tile([128, C_CORE], F32)
            for j in range(NB_CORE):
                nc.tensor.matmul(
                    mp[:, j * 128:(j + 1) * 128],
                    xt[:, j * 128:(j + 1) * 128],
                    w_sb[:, j * 128:(j + 1) * 128],
                    start=(j == 0),
                    stop=(j == NB_CORE - 1),
                )
            gi = (g % G_PER_OUT) * C_CORE
            nc.vector.tensor_add(out_tile[:, gi:gi + C_CORE], mp, bias_sb)
            # Stores: full [128, 4096] tiles on the Sync ring, except the
            # last two tiles which go out per-group on alternating rings
            # (loads are drained by then) so the tail is one 256 KiB piece.
            if g >= GROUPS - 2 * G_PER_OUT:
                eng = nc.sync if g % 2 == 0 else nc.scalar
                cols = slice(g * C_CORE, (g + 1) * C_CORE)
                eng.dma_start(out=o_d[:, cols], in_=out_tile[:, gi:gi + C_CORE])
            elif g % G_PER_OUT == G_PER_OUT - 1:
                t = g // G_PER_OUT
                cols = slice(t * G_PER_OUT * C_CORE, (t + 1) * G_PER_OUT * C_CORE)
                nc.sync.dma_start(out=o_d[:, cols], in_=out_tile)

    nc.compile()
    return nc


def _get_nc():
    if "nc" not in _NC_CACHE:
        _NC_CACHE["nc"] = _build_nc()
    return _NC_CACHE["nc"]


def _run(inputs, trace=False):
    x = np.asarray(inputs["x"], dtype=np.float32)
    weights = np.asarray(inputs["weights"], dtype=np.float32)
    bias = np.asarray(inputs["bias"], dtype=np.float32)
    orig_shape = x.shape
    xf = x.reshape(B_FULL, SIZE)

    nc = _get_nc()
    in_maps = []
    for i in range(N_CORES):
        cols = slice(i * C_CORE, (i + 1) * C_CORE)
        # pack: xp[p, g*512 + c] = xf[g*128 + p, 512*i + c]
        xp = np.ascontiguousarray(
            xf[:, cols].reshape(GROUPS, 128, C_CORE).transpose(1, 0, 2)
            .reshape(128, XP_COLS)
        )
        # weights d-major per core: [d, j*128+e] = W[4i+j, d, e], cast bf16
        w_t = np.ascontiguousarray(
            weights[i * NB_CORE:(i + 1) * NB_CORE].transpose(1, 0, 2)
            .reshape(BLK, C_CORE)
        ).astype(ml_dtypes.bfloat16)
        b_rep = np.ascontiguousarray(
            np.broadcast_to(bias[cols][None, :], (128, C_CORE))
        )
        in_maps.append({"x": xp, "weights": w_t, "bias": b_rep})
    res = run_bass_kernel_spmd(
        nc, in_maps, core_ids=list(range(N_CORES)), trace=trace
    )
    out = np.empty((B_FULL, SIZE), dtype=np.float32)
    for i in range(N_CORES):
        cols = slice(i * C_CORE, (i + 1) * C_CORE)
        op = res.results[i]["out"]
        out[:, cols] = (
            op.reshape(128, GROUPS, C_CORE).transpose(1, 0, 2)
            .reshape(B_FULL, C_CORE)
        )
    return out.reshape(orig_shape), res


def kernel(**inputs):
    out, _ = _run(inputs, trace=False)
    return out


# revision 6
# speedup vs baseline: 1.1604x; 1.0672x over previous
"""Block-diagonal MLP kernel for Trainium2 (8 NeuronCores, expert-sharded).

Computes out = blockdiag_matmul(x, weights) + bias where
  x: [4, 2048, 4096] f32, weights: [32, 128, 128] f32, bias: [4096] f32.

Strategy: shard the 32 diagonal blocks across 8 cores (4 blocks = 512
feature columns each); every core sees all 8192 flattened rows of its
512-column slice.  Per-core DMA is 16.78 MB in + 16.78 MB out + 0.2 MB
consts.  Loads and stores co-flow on the two HWDGE rings, which
together sustain ~430 GB/s (the SBUF-AXI fabric ceiling) -- the body
floor is ~79 us, so the schedule's whole job is to avoid solo-load /
solo-store phases that cap at ~240-340 GB/s.

The host packs each core's x shard as [128, 32768] (partition p holds
the rows congruent to p mod 128, 64 row-groups side by side), so DMA
per-partition lines are 8 KiB (2 KiB descriptors measured ~35% slower
per ring).  x streams through a rotating pool of [128, 2048] chunk
buffers (4 groups each): each load waits for the transposes of the
chunk 4 buffers back, so loads self-pace to compute rate instead of
front-loading, and stores (ready from ~14 us) overlap loads for the
whole body.  Loads ride the ACT ring, stores the Sync ring; the last
two out-tiles store per-group alternating across both rings so the
tail drains at both-ring rate.

Per 512-column group: PE transpose-mode matmuls (fp32) put the
contraction dim on partitions; ACT evacuates the transpose to SBUF
casting fp32->bf16 (free cast -- bf16 halves the real matmul cost);
bf16 matmuls against SBUF-resident bf16 weights (host-cast, the same
4 blocks for all 64 groups); DVE evacuates with the bias add fused.
Transposes run two groups ahead of the consuming matmuls.  The bias
[1,512] row is broadcast to 128 partitions once on-chip via a K=1
ones-matmul.  bf16 is only used for matmul operands (fp32 PSUM
accumulation); max rel err vs the fp32 reference ~2e-3 (gate 2e-2).
"""
import numpy as np
import ml_dtypes
from contextlib import ExitStack

import concourse.mybir as mybir
import concourse.tile as tile
from concourse import bacc
from concourse.bass_utils import run_bass_kernel_spmd

F32 = mybir.dt.float32
BF16 = mybir.dt.bfloat16

SIZE = 4096
NB = 32            # number of diagonal blocks
BLK = 128          # block size
N_CORES = 8
NB_CORE = NB // N_CORES        # 4 blocks per core
C_CORE = NB_CORE * BLK         # 512 feature columns per core
B_FULL = 4 * 2048              # 8192 flattened rows (all on every core)
GROUPS = B_FULL // 128         # 64 row-groups of [128, 512]
XP_COLS = GROUPS * C_CORE      # 32768 packed columns
G_PER_CHUNK = 4                # groups per load chunk [128, 2048]
N_CHUNKS = GROUPS // G_PER_CHUNK
G_PER_OUT = 4                  # groups per store tile [128, 2048]
TAIL_GROUPS = 8                # last groups stored per-group on both rings

_NC_CACHE = {}


def _build_nc():
    nc = bacc.Bacc()
    x_d = nc.declare_dram_parameter("x", [128, XP_COLS], F32, isOutput=False)
    w_d = nc.declare_dram_parameter("weights", [BLK, C_CORE], BF16, isOutput=False)
    b_d = nc.declare_dram_parameter("bias", [1, C_CORE], F32, isOutput=False)
    i_d = nc.declare_dram_parameter("ident", [BLK, BLK], F32, isOutput=False)
    n_d = nc.declare_dram_parameter("ones", [1, BLK], F32, isOutput=False)
    o_d = nc.declare_dram_parameter("out", [128, XP_COLS], F32, isOutput=True)

    with tile.TileContext(nc) as tc, ExitStack() as ctx:
        consts = ctx.enter_context(tc.tile_pool(name="consts", bufs=1))
        x_pool = ctx.enter_context(tc.tile_pool(name="x", bufs=4))
        xt_pool = ctx.enter_context(tc.tile_pool(name="xt", bufs=3))
        out_pool = ctx.enter_context(tc.tile_pool(name="out", bufs=3))
        tp_pool = ctx.enter_context(tc.tile_pool(name="tp", bufs=3, space="PSUM"))
        mp_pool = ctx.enter_context(tc.tile_pool(name="mp", bufs=3, space="PSUM"))
        bp_pool = ctx.enter_context(tc.tile_pool(name="bp", bufs=1, space="PSUM"))

        ident = consts.tile([BLK, BLK], F32)
        ones = consts.tile([1, BLK], F32)
        w_sb = consts.tile([BLK, C_CORE], BF16)
        b_row = consts.tile([1, C_CORE], F32)
        bias_sb = consts.tile([128, C_CORE], F32)

        # Consts: identity (needed by the first transpose ~10.5 us in)
        # leads the Sync ring; weights/bias lead the ACT ring ahead of
        # the x stream.
        nc.sync.dma_start(out=ident, in_=i_d[:, :])
        nc.sync.dma_start(out=ones, in_=n_d[:, :])
        nc.scalar.dma_start(out=w_sb, in_=w_d[:, :])
        nc.scalar.dma_start(out=b_row, in_=b_d[:, :])

        # Broadcast bias across partitions: [128,512] = ones.T @ b_row.
        bp = bp_pool.tile([128, C_CORE], F32)
        nc.tensor.matmul(bp, ones, b_row, start=True, stop=True)
        nc.vector.tensor_copy(bias_sb, bp)

        x_chunks = [None] * N_CHUNKS

        def emit_load(c):
            xc = x_pool.tile([128, G_PER_CHUNK * C_CORE], F32)
            base = c * G_PER_CHUNK * C_CORE
            if c == 0:
                # split so the first transposes start half a transfer sooner
                nc.scalar.dma_start(out=xc[:, 0:512], in_=x_d[:, 0:512])
                nc.scalar.dma_start(out=xc[:, 512:2048], in_=x_d[:, base + 512:base + 2048])
            else:
                nc.scalar.dma_start(out=xc, in_=x_d[:, base:base + G_PER_CHUNK * C_CORE])
            x_chunks[c] = xc

        for c in range(3):
            emit_load(c)

        def emit_transposes(g):
            tp = tp_pool.tile([128, C_CORE], F32)
            xc = x_chunks[g // G_PER_CHUNK]
            gb = (g % G_PER_CHUNK) * C_CORE
            for j in range(NB_CORE):
                nc.tensor.matmul(
                    tp[:, j * 128:(j + 1) * 128],
                    xc[:, gb + j * 128:gb + (j + 1) * 128],
                    ident,
                    is_transpose=True,
                    start=(j == 0),
                    stop=(j == NB_CORE - 1),
                )
            xt = xt_pool.tile([128, C_CORE], BF16)
            nc.scalar.copy(xt, tp)   # PSUM f32 -> SBUF bf16
            return xt

        xt_q = [emit_transposes(0), emit_transposes(1)]
        out_tile = None
        for g in range(GROUPS):
            if g % G_PER_OUT == 0:
                out_tile = out_pool.tile([128, G_PER_OUT * C_CORE], F32)
            # prefetch: 3 chunks (12 groups) ahead of the transposes,
            # which themselves run 2 groups ahead of the matmuls here
            if g % G_PER_CHUNK == 0 and (gc := g // G_PER_CHUNK + 3) < N_CHUNKS:
                emit_load(gc)
            xt = xt_q.pop(0)
            if g + 2 < GROUPS:
                xt_q.append(emit_transposes(g + 2))
            mp = mp_pool.tile([128, C_CORE], F32)
            for j in range(NB_CORE):
                nc.tensor.matmul(
                    mp[:, j * 128:(j + 1) * 128],
                    xt[:, j * 128:(j + 1) * 128],
                    w_sb[:, j * 128:(j + 1) * 128],
                    start=(j == 0),
                    stop=(j == NB_CORE - 1),
                )
            gi = (g % G_PER_OUT) * C_CORE
            nc.vector.tensor_add(out_tile[:, gi:gi + C_CORE], mp, bias_sb)
            if g >= GROUPS - TAIL_GROUPS:
                # loads are drained by now: store per-group alternating
                # across both rings so the tail empties at both-ring rate.
                eng = nc.sync if g % 2 == 0 else nc.scalar
                cols = slice(g * C_CORE, (g + 1) * C_CORE)
                eng.dma_start(out=o_d[:, cols], in_=out_tile[:, gi:gi + C_CORE])
            elif g % G_PER_OUT == G_PER_OUT - 1:
                t = g // G_PER_OUT
                cols = slice(t * G_PER_OUT * C_CORE, (t + 1) * G_PER_OUT * C_CORE)
                nc.sync.dma_start(out=o_d[:, cols], in_=out_tile)

    nc.compile()
    return nc


def _get_nc():
    if "nc" not in _NC_CACHE:
        _NC_CACHE["nc"] = _build_nc()
    return _NC_CACHE["nc"]


def _run(inputs, trace=False):
    x = np.asarray(inputs["x"], dtype=np.float32)
    weights = np.asarray(inputs["weights"], dtype=np.float32)
    bias = np.asarray(inputs["bias"], dtype=np.float32)
    orig_shape = x.shape
    xf = x.reshape(B_FULL, SIZE)
    ident = np.eye(BLK, dtype=np.float32)
    ones = np.ones((1, BLK), dtype=np.float32)

    nc = _get_nc()
    in_maps = []
    for i in range(N_CORES):
        cols = slice(i * C_CORE, (i + 1) * C_CORE)
        # pack: xp[p, g*512 + c] = xf[g*128 + p, 512*i + c]
        xp = np.ascontiguousarray(
            xf[:, cols].reshape(GROUPS, 128, C_CORE).transpose(1, 0, 2)
            .reshape(128, XP_COLS)
        )
        # weights d-major per core: [d, j*128+e] = W[4i+j, d, e], cast bf16
        w_t = np.ascontiguousarray(
            weights[i * NB_CORE:(i + 1) * NB_CORE].transpose(1, 0, 2)
            .reshape(BLK, C_CORE)
        ).astype(ml_dtypes.bfloat16)
        in_maps.append(
            {
                "x": xp,
                "weights": w_t,
                "bias": np.ascontiguousarray(bias[cols][None, :]),
                "ident": ident,
                "ones": ones,
            }
        )
    res = run_bass_kernel_spmd(
        nc, in_maps, core_ids=list(range(N_CORES)), trace=trace
    )
    out = np.empty((B_FULL, SIZE), dtype=np.float32)
    for i in range(N_CORES):
        cols = slice(i * C_CORE, (i + 1) * C_CORE)
        op = res.results[i]["out"]
        out[:, cols] = (
            op.reshape(128, GROUPS, C_CORE).transpose(1, 0, 2)
            .reshape(B_FULL, C_CORE)
        )
    return out.reshape(orig_shape), res


def kernel(**inputs):
    out, _ = _run(inputs, trace=False)
    return out


# revision 12
# speedup vs baseline: 1.1689x; 1.0073x over previous
"""Block-diagonal MLP kernel for Trainium2 (8 NeuronCores, expert-sharded).

Computes out = blockdiag_matmul(x, weights) + bias where
  x: [4, 2048, 4096] f32, weights: [32, 128, 128] f32, bias: [4096] f32.

Strategy: shard the 32 diagonal blocks across 8 cores (4 blocks = 512
feature columns each); every core sees all 8192 flattened rows of its
512-column slice.  Per-core DMA is 16.78 MB in + 16.78 MB out + 0.2 MB
consts.  Loads and stores co-flow on the two HWDGE rings, which
together sustain ~430 GB/s (the SBUF-AXI fabric ceiling) -- the body
floor is ~79 us, so the schedule's whole job is to avoid solo-load /
solo-store phases that cap at ~240-340 GB/s.

The host packs each core's x shard as [128, 32768] (partition p holds
the rows congruent to p mod 128, 64 row-groups side by side), so DMA
per-partition lines are 8 KiB (2 KiB descriptors measured ~35% slower
per ring).  x streams through a rotating pool of [128, 2048] chunk
buffers (4 groups each): each load waits for the transposes of the
chunk 4 buffers back, so loads self-pace to compute rate instead of
front-loading, and stores (ready from ~14 us) overlap loads for the
whole body.  Loads ride the ACT ring, stores the Sync ring; the last
two out-tiles store per-group alternating across both rings so the
tail drains at both-ring rate.

Per 512-column group: PE transpose-mode matmuls (fp32) put the
contraction dim on partitions; ACT evacuates the transpose to SBUF
casting fp32->bf16 (free cast -- bf16 halves the real matmul cost);
bf16 matmuls against SBUF-resident bf16 weights (host-cast, the same
4 blocks for all 64 groups); DVE evacuates with the bias add fused.
Transposes run two groups ahead of the consuming matmuls.  The bias
[1,512] row is broadcast to 128 partitions once on-chip via a K=1
ones-matmul.  bf16 is only used for matmul operands (fp32 PSUM
accumulation); max rel err vs the fp32 reference ~2e-3 (gate 2e-2).
"""
import numpy as np
import ml_dtypes
from contextlib import ExitStack

import concourse.mybir as mybir
import concourse.tile as tile
from concourse import bacc
from concourse.bass_utils import run_bass_kernel_spmd

F32 = mybir.dt.float32
BF16 = mybir.dt.bfloat16

SIZE = 4096
NB = 32            # number of diagonal blocks
BLK = 128          # block size
N_CORES = 8
NB_CORE = NB // N_CORES        # 4 blocks per core
C_CORE = NB_CORE * BLK         # 512 feature columns per core
B_FULL = 4 * 2048              # 8192 flattened rows (all on every core)
GROUPS = B_FULL // 128         # 64 row-groups of [128, 512]
XP_COLS = GROUPS * C_CORE      # 32768 packed columns
G_PER_CHUNK = 4                # groups per load chunk [128, 2048]
N_CHUNKS = GROUPS // G_PER_CHUNK
G_PER_OUT = 4                  # groups per store tile [128, 2048]
TAIL_GROUPS = 8                # last groups stored per-group on both rings

_NC_CACHE = {}


def _build_nc():
    nc = bacc.Bacc()
    x_d = nc.declare_dram_parameter("x", [128, XP_COLS], F32, isOutput=False)
    w_d = nc.declare_dram_parameter("weights", [BLK, C_CORE], BF16, isOutput=False)
    b_d = nc.declare_dram_parameter("bias", [1, C_CORE], F32, isOutput=False)
    i_d = nc.declare_dram_parameter("ident", [BLK, BLK], F32, isOutput=False)
    n_d = nc.declare_dram_parameter("ones", [1, BLK], F32, isOutput=False)
    o_d = nc.declare_dram_parameter("out", [128, XP_COLS], F32, isOutput=True)

    with tile.TileContext(nc) as tc, ExitStack() as ctx:
        consts = ctx.enter_context(tc.tile_pool(name="consts", bufs=1))
        x_pool = ctx.enter_context(tc.tile_pool(name="x", bufs=6))
        xt_pool = ctx.enter_context(tc.tile_pool(name="xt", bufs=4))
        out_pool = ctx.enter_context(tc.tile_pool(name="out", bufs=4))
        tp_pool = ctx.enter_context(tc.tile_pool(name="tp", bufs=3, space="PSUM"))
        mp_pool = ctx.enter_context(tc.tile_pool(name="mp", bufs=3, space="PSUM"))
        bp_pool = ctx.enter_context(tc.tile_pool(name="bp", bufs=1, space="PSUM"))

        ident = consts.tile([BLK, BLK], F32)
        ones = consts.tile([1, BLK], F32)
        w_sb = consts.tile([BLK, C_CORE], BF16)
        b_row = consts.tile([1, C_CORE], F32)
        bias_sb = consts.tile([128, C_CORE], F32)

        # Consts: identity (needed by the first transpose ~10.5 us in)
        # leads the Sync ring; weights/bias lead the ACT ring ahead of
        # the x stream.
        nc.sync.dma_start(out=ident, in_=i_d[:, :])
        nc.sync.dma_start(out=ones, in_=n_d[:, :])
        nc.scalar.dma_start(out=w_sb, in_=w_d[:, :])
        nc.scalar.dma_start(out=b_row, in_=b_d[:, :])

        # Broadcast bias across partitions: [128,512] = ones.T @ b_row.
        bp = bp_pool.tile([128, C_CORE], F32)
        nc.tensor.matmul(bp, ones, b_row, start=True, stop=True)
        nc.vector.tensor_copy(bias_sb, bp)

        x_chunks = [None] * N_CHUNKS

        def emit_load(c):
            xc = x_pool.tile([128, G_PER_CHUNK * C_CORE], F32)
            base = c * G_PER_CHUNK * C_CORE
            if c == 0:
                # split so the first transposes start half a transfer sooner
                nc.scalar.dma_start(out=xc[:, 0:512], in_=x_d[:, 0:512])
                nc.scalar.dma_start(out=xc[:, 512:2048], in_=x_d[:, base + 512:base + 2048])
            else:
                nc.scalar.dma_start(out=xc, in_=x_d[:, base:base + G_PER_CHUNK * C_CORE])
            x_chunks[c] = xc

        for c in range(4):
            emit_load(c)

        def emit_transposes(g):
            tp = tp_pool.tile([128, C_CORE], F32)
            xc = x_chunks[g // G_PER_CHUNK]
            gb = (g % G_PER_CHUNK) * C_CORE
            for j in range(NB_CORE):
                nc.tensor.matmul(
                    tp[:, j * 128:(j + 1) * 128],
                    xc[:, gb + j * 128:gb + (j + 1) * 128],
                    ident,
                    is_transpose=True,
                    start=(j == 0),
                    stop=(j == NB_CORE - 1),
                )
            xt = xt_pool.tile([128, C_CORE], BF16)
            nc.scalar.copy(xt, tp)   # PSUM f32 -> SBUF bf16
            return xt

        xt_q = [emit_transposes(0), emit_transposes(1)]
        out_tile = None
        for g in range(GROUPS):
            if g % G_PER_OUT == 0:
                out_tile = out_pool.tile([128, G_PER_OUT * C_CORE], F32)
            # prefetch: 4 chunks (16 groups) ahead of the transposes,
            # which themselves run 2 groups ahead of the matmuls here
            if g % G_PER_CHUNK == 0 and (gc := g // G_PER_CHUNK + 4) < N_CHUNKS:
                emit_load(gc)
            xt = xt_q.pop(0)
            if g + 2 < GROUPS:
                xt_q.append(emit_transposes(g + 2))
            mp = mp_pool.tile([128, C_CORE], F32)
            for j in range(NB_CORE):
                nc.tensor.matmul(
                    mp[:, j * 128:(j + 1) * 128],
                    xt[:, j * 128:(j + 1) * 128],
                    w_sb[:, j * 128:(j + 1) * 128],
                    start=(j == 0),
                    stop=(j == NB_CORE - 1),
                )
            gi = (g % G_PER_OUT) * C_CORE
            nc.vector.tensor_add(out_tile[:, gi:gi + C_CORE], mp, bias_sb)
            if g >= GROUPS - TAIL_GROUPS:
                # loads are drained by now: store per-group alternating
                # across both rings so the tail empties at both-ring rate.
                eng = nc.sync if g % 2 == 0 else nc.scalar
                cols = slice(g * C_CORE, (g + 1) * C_CORE)
                eng.dma_start(out=o_d[:, cols], in_=out_tile[:, gi:gi + C_CORE])
            elif g % G_PER_OUT == G_PER_OUT - 1:
                t = g // G_PER_OUT
                cols = slice(t * G_PER_OUT * C_CORE, (t + 1) * G_PER_OUT * C_CORE)
                nc.sync.dma_start(out=o_d[:, cols], in_=out_tile)

    nc.compile()
    return nc


def _get_nc():
    if "nc" not in _NC_CACHE:
        _NC_CACHE["nc"] = _build_nc()
    return _NC_CACHE["nc"]


def _run(inputs, trace=False):
    x = np.asarray(inputs["x"], dtype=np.float32)
    weights = np.asarray(inputs["weights"], dtype=np.float32)
    bias = np.asarray(inputs["bias"], dtype=np.float32)
    orig_shape = x.shape
    xf = x.reshape(B_FULL, SIZE)
    ident = np.eye(BLK, dtype=np.float32)
    ones = np.ones((1, BLK), dtype=np.float32)

    nc = _get_nc()
    in_maps = []
    for i in range(N_CORES):
        cols = slice(i * C_CORE, (i + 1) * C_CORE)
        # pack: xp[p, g*512 + c] = xf[g*128 + p, 512*i + c]
        xp = np.ascontiguousarray(
            xf[:, cols].reshape(GROUPS, 128, C_CORE).transpose(1, 0, 2)
            .reshape(128, XP_COLS)
        )
        # weights d-major per core: [d, j*128+e] = W[4i+j, d, e], cast bf16
        w_t = np.ascontiguousarray(
            weights[i * NB_CORE:(i + 1) * NB_CORE].transpose(1, 0, 2)
            .reshape(BLK, C_CORE)
        ).astype(ml_dtypes.bfloat16)
        in_maps.append(
            {
                "x": xp,
                "weights": w_t,
                "bias": np.ascontiguousarray(bias[cols][None, :]),
                "ident": ident,
                "ones": ones,
            }
        )
    res = run_bass_kernel_spmd(
        nc, in_maps, core_ids=list(range(N_CORES)), trace=trace
    )
    out = np.empty((B_FULL, SIZE), dtype=np.float32)
    for i in range(N_CORES):
        cols = slice(i * C_CORE, (i + 1) * C_CORE)
        op = res.results[i]["out"]
        out[:, cols] = (
            op.reshape(128, GROUPS, C_CORE).transpose(1, 0, 2)
            .reshape(B_FULL, C_CORE)
        )
    return out.reshape(orig_shape), res


def kernel(**inputs):
    out, _ = _run(inputs, trace=False)
    return out


# revision 20
# speedup vs baseline: 1.1922x; 1.0200x over previous
"""Block-diagonal MLP kernel for Trainium2 (8 NeuronCores, expert-sharded).

Computes out = blockdiag_matmul(x, weights) + bias where
  x: [4, 2048, 4096] f32, weights: [32, 128, 128] f32, bias: [4096] f32.

Strategy: shard the 32 diagonal blocks across 8 cores (4 blocks = 512
feature columns each); every core sees all 8192 flattened rows of its
512-column slice.  Per-core DMA is 16.78 MB in + 16.78 MB out + 0.2 MB
consts.  Loads and stores co-flow on the two HWDGE rings, which
together sustain ~430 GB/s (the SBUF-AXI fabric ceiling) -- the body
floor is ~79 us, so the schedule's whole job is to avoid solo-load /
solo-store phases that cap at ~240-340 GB/s.

The host packs each core's x shard as [128, 32768] (partition p holds
the rows congruent to p mod 128, 64 row-groups side by side), so DMA
per-partition lines are 8 KiB (2 KiB descriptors measured ~35% slower
per ring).  x streams through a rotating pool of [128, 2048] chunk
buffers (4 groups each): each load waits for the transposes of the
chunk 4 buffers back, so loads self-pace to compute rate instead of
front-loading, and stores (ready from ~14 us) overlap loads for the
whole body.  Loads ride the ACT ring, stores the Sync ring; the last
two out-tiles store per-group alternating across both rings so the
tail drains at both-ring rate.

Per 512-column group: PE transpose-mode matmuls (fp32) put the
contraction dim on partitions; ACT evacuates the transpose to SBUF
casting fp32->bf16 (free cast -- bf16 halves the real matmul cost);
bf16 matmuls against SBUF-resident bf16 weights (host-cast, the same
4 blocks for all 64 groups); DVE evacuates with the bias add fused.
Transposes run two groups ahead of the consuming matmuls.  The bias
[1,512] row is broadcast to 128 partitions once on-chip via a K=1
ones-matmul.  bf16 is only used for matmul operands (fp32 PSUM
accumulation); max rel err vs the fp32 reference ~2e-3 (gate 2e-2).
"""
import numpy as np
import ml_dtypes
from contextlib import ExitStack

import concourse.mybir as mybir
import concourse.tile as tile
from concourse import bacc
from concourse.bass_utils import run_bass_kernel_spmd

F32 = mybir.dt.float32
BF16 = mybir.dt.bfloat16

SIZE = 4096
NB = 32            # number of diagonal blocks
BLK = 128          # block size
N_CORES = 8
NB_CORE = NB // N_CORES        # 4 blocks per core
C_CORE = NB_CORE * BLK         # 512 feature columns per core
B_FULL = 4 * 2048              # 8192 flattened rows (all on every core)
GROUPS = B_FULL // 128         # 64 row-groups of [128, 512]
XP_COLS = GROUPS * C_CORE      # 32768 packed columns
G_PER_CHUNK = 8                # groups per load chunk [128, 4096]
N_CHUNKS = GROUPS // G_PER_CHUNK
G_PER_OUT = 4                  # groups per store tile [128, 2048]
TAIL_GROUPS = 8                # last groups stored per-group on both rings

_NC_CACHE = {}


def _build_nc():
    nc = bacc.Bacc()
    x_d = nc.declare_dram_parameter("x", [128, XP_COLS], F32, isOutput=False)
    w_d = nc.declare_dram_parameter("weights", [BLK, C_CORE], BF16, isOutput=False)
    b_d = nc.declare_dram_parameter("bias", [1, C_CORE], F32, isOutput=False)
    i_d = nc.declare_dram_parameter("ident", [BLK, BLK], BF16, isOutput=False)
    n_d = nc.declare_dram_parameter("ones", [1, BLK], F32, isOutput=False)
    o_d = nc.declare_dram_parameter("out", [128, XP_COLS], F32, isOutput=True)

    with tile.TileContext(nc) as tc, ExitStack() as ctx:
        consts = ctx.enter_context(tc.tile_pool(name="consts", bufs=1))
        x_pool = ctx.enter_context(tc.tile_pool(name="x", bufs=3))
        xt_pool = ctx.enter_context(tc.tile_pool(name="xt", bufs=4))
        out_pool = ctx.enter_context(tc.tile_pool(name="out", bufs=4))
        tp_pool = ctx.enter_context(tc.tile_pool(name="tp", bufs=3, space="PSUM"))
        mp_pool = ctx.enter_context(tc.tile_pool(name="mp", bufs=3, space="PSUM"))
        bp_pool = ctx.enter_context(tc.tile_pool(name="bp", bufs=1, space="PSUM"))

        ident = consts.tile([BLK, BLK], BF16)
        ones = consts.tile([1, BLK], F32)
        w_sb = consts.tile([BLK, C_CORE], BF16)
        b_row = consts.tile([1, C_CORE], F32)
        bias_sb = consts.tile([128, C_CORE], F32)

        # Consts: identity (needed by the first transpose ~10.5 us in)
        # leads the Sync ring; weights/bias lead the ACT ring ahead of
        # the x stream.
        nc.sync.dma_start(out=ident, in_=i_d[:, :])
        nc.sync.dma_start(out=ones, in_=n_d[:, :])
        nc.scalar.dma_start(out=w_sb, in_=w_d[:, :])
        nc.scalar.dma_start(out=b_row, in_=b_d[:, :])

        # Broadcast bias across partitions: [128,512] = ones.T @ b_row.
        bp = bp_pool.tile([128, C_CORE], F32)
        nc.tensor.matmul(bp, ones, b_row, start=True, stop=True)
        nc.vector.tensor_copy(bias_sb, bp)

        x_chunks = [None] * N_CHUNKS

        def emit_load(c):
            # SWDGE (gpsimd) DMA casts f32 DRAM -> bf16 SBUF inline in the
            # DMA engines: halves the SBUF-fabric bytes on the load side
            # and halves the PE transpose cost, for free.
            xc = x_pool.tile([128, G_PER_CHUNK * C_CORE], BF16)
            base = c * G_PER_CHUNK * C_CORE
            if c == 0:
                # split so the first transposes start earlier
                nc.gpsimd.dma_start(out=xc[:, 0:512], in_=x_d[:, 0:512])
                nc.gpsimd.dma_start(out=xc[:, 512:2048], in_=x_d[:, 512:2048])
                nc.gpsimd.dma_start(out=xc[:, 2048:4096], in_=x_d[:, 2048:4096])
            else:
                nc.gpsimd.dma_start(out=xc, in_=x_d[:, base:base + G_PER_CHUNK * C_CORE])
            x_chunks[c] = xc

        for c in range(2):
            emit_load(c)

        def emit_transposes(g):
            tp = tp_pool.tile([128, C_CORE], BF16)
            xc = x_chunks[g // G_PER_CHUNK]
            gb = (g % G_PER_CHUNK) * C_CORE
            for j in range(NB_CORE):
                nc.tensor.matmul(
                    tp[:, j * 128:(j + 1) * 128],
                    xc[:, gb + j * 128:gb + (j + 1) * 128],
                    ident,
                    is_transpose=True,
                    start=(j == 0),
                    stop=(j == NB_CORE - 1),
                )
            xt = xt_pool.tile([128, C_CORE], BF16)
            nc.scalar.copy(xt, tp)   # PSUM f32 -> SBUF bf16
            return xt

        xt_q = [emit_transposes(0), emit_transposes(1)]
        out_tile = None
        for g in range(GROUPS):
            if g % G_PER_OUT == 0:
                out_tile = out_pool.tile([128, G_PER_OUT * C_CORE], F32)
            # prefetch: 2 chunks (16 groups) ahead of the transposes,
            # which themselves run 2 groups ahead of the matmuls here
            if g % G_PER_CHUNK == 0 and (gc := g // G_PER_CHUNK + 2) < N_CHUNKS:
                emit_load(gc)
            xt = xt_q.pop(0)
            if g + 2 < GROUPS:
                xt_q.append(emit_transposes(g + 2))
            mp = mp_pool.tile([128, C_CORE], F32)
            for j in range(NB_CORE):
                nc.tensor.matmul(
                    mp[:, j * 128:(j + 1) * 128],
                    xt[:, j * 128:(j + 1) * 128],
                    w_sb[:, j * 128:(j + 1) * 128],
                    start=(j == 0),
                    stop=(j == NB_CORE - 1),
                )
            gi = (g % G_PER_OUT) * C_CORE
            nc.vector.tensor_add(out_tile[:, gi:gi + C_CORE], mp, bias_sb)
            if g >= GROUPS - TAIL_GROUPS:
                # loads are drained by now: store per-group alternating
                # across both rings so the tail empties at both-ring rate.
                eng = nc.sync if g % 2 == 0 else nc.scalar
                cols = slice(g * C_CORE, (g + 1) * C_CORE)
                eng.dma_start(out=o_d[:, cols], in_=out_tile[:, gi:gi + C_CORE])
            elif g % G_PER_OUT == G_PER_OUT - 1:
                t = g // G_PER_OUT
                cols = slice(t * G_PER_OUT * C_CORE, (t + 1) * G_PER_OUT * C_CORE)
                nc.sync.dma_start(out=o_d[:, cols], in_=out_tile)

    nc.compile()
    return nc


def _get_nc():
    if "nc" not in _NC_CACHE:
        _NC_CACHE["nc"] = _build_nc()
    return _NC_CACHE["nc"]


def _run(inputs, trace=False):
    x = np.asarray(inputs["x"], dtype=np.float32)
    weights = np.asarray(inputs["weights"], dtype=np.float32)
    bias = np.asarray(inputs["bias"], dtype=np.float32)
    orig_shape = x.shape
    xf = x.reshape(B_FULL, SIZE)
    ident = np.eye(BLK, dtype=np.float32).astype(ml_dtypes.bfloat16)
    ones = np.ones((1, BLK), dtype=np.float32)

    nc = _get_nc()
    in_maps = []
    for i in range(N_CORES):
        cols = slice(i * C_CORE, (i + 1) * C_CORE)
        # pack: xp[p, g*512 + c] = xf[g*128 + p, 512*i + c]
        xp = np.ascontiguousarray(
            xf[:, cols].reshape(GROUPS, 128, C_CORE).transpose(1, 0, 2)
            .reshape(128, XP_COLS)
        )
        # weights d-major per core: [d, j*128+e] = W[4i+j, d, e], cast bf16
        w_t = np.ascontiguousarray(
            weights[i * NB_CORE:(i + 1) * NB_CORE].transpose(1, 0, 2)
            .reshape(BLK, C_CORE)
        ).astype(ml_dtypes.bfloat16)
        in_maps.append(
            {
                "x": xp,
                "weights": w_t,
                "bias": np.ascontiguousarray(bias[cols][None, :]),
                "ident": ident,
                "ones": ones,
            }
        )
    res = run_bass_kernel_spmd(
        nc, in_maps, core_ids=list(range(N_CORES)), trace=trace
    )
    out = np.empty((B_FULL, SIZE), dtype=np.float32)
    for i in range(N_CORES):
        cols = slice(i * C_CORE, (i + 1) * C_CORE)
        op = res.results[i]["out"]
        out[:, cols] = (
            op.reshape(128, GROUPS, C_CORE).transpose(1, 0, 2)
            .reshape(B_FULL, C_CORE)
        )
    return out.reshape(orig_shape), res


def kernel(**inputs):
    out, _ = _run(inputs, trace=False)
    return out


# revision 21
# speedup vs baseline: 1.2070x; 1.0124x over previous
"""Block-diagonal MLP kernel for Trainium2 (8 NeuronCores, expert-sharded).

Computes out = blockdiag_matmul(x, weights) + bias where
  x: [4, 2048, 4096] f32, weights: [32, 128, 128] f32, bias: [4096] f32.

Strategy: shard the 32 diagonal blocks across 8 cores (4 blocks = 512
feature columns each); every core sees all 8192 flattened rows of its
512-column slice.  Per-core DMA is 16.78 MB in + 16.78 MB out + 0.2 MB
consts.  Loads and stores co-flow on the two HWDGE rings, which
together sustain ~430 GB/s (the SBUF-AXI fabric ceiling) -- the body
floor is ~79 us, so the schedule's whole job is to avoid solo-load /
solo-store phases that cap at ~240-340 GB/s.

The host packs each core's x shard as [128, 32768] (partition p holds
the rows congruent to p mod 128, 64 row-groups side by side), so DMA
per-partition lines are 8 KiB (2 KiB descriptors measured ~35% slower
per ring).  x streams through a rotating pool of [128, 2048] chunk
buffers (4 groups each): each load waits for the transposes of the
chunk 4 buffers back, so loads self-pace to compute rate instead of
front-loading, and stores (ready from ~14 us) overlap loads for the
whole body.  Loads ride the ACT ring, stores the Sync ring; the last
two out-tiles store per-group alternating across both rings so the
tail drains at both-ring rate.

Per 512-column group: PE transpose-mode matmuls (fp32) put the
contraction dim on partitions; ACT evacuates the transpose to SBUF
casting fp32->bf16 (free cast -- bf16 halves the real matmul cost);
bf16 matmuls against SBUF-resident bf16 weights (host-cast, the same
4 blocks for all 64 groups); DVE evacuates with the bias add fused.
Transposes run two groups ahead of the consuming matmuls.  The bias
[1,512] row is broadcast to 128 partitions once on-chip via a K=1
ones-matmul.  bf16 is only used for matmul operands (fp32 PSUM
accumulation); max rel err vs the fp32 reference ~2e-3 (gate 2e-2).
"""
import numpy as np
import ml_dtypes
from contextlib import ExitStack

import concourse.mybir as mybir
import concourse.tile as tile
from concourse import bacc
from concourse.bass_utils import run_bass_kernel_spmd

F32 = mybir.dt.float32
BF16 = mybir.dt.bfloat16

SIZE = 4096
NB = 32            # number of diagonal blocks
BLK = 128          # block size
N_CORES = 8
NB_CORE = NB // N_CORES        # 4 blocks per core
C_CORE = NB_CORE * BLK         # 512 feature columns per core
B_FULL = 4 * 2048              # 8192 flattened rows (all on every core)
GROUPS = B_FULL // 128         # 64 row-groups of [128, 512]
XP_COLS = GROUPS * C_CORE      # 32768 packed columns
G_PER_CHUNK = 8                # groups per load chunk [128, 4096]
N_CHUNKS = GROUPS // G_PER_CHUNK
G_PER_OUT = 4                  # groups per store tile [128, 2048]
TAIL_GROUPS = 8                # last groups stored per-group on both rings

_NC_CACHE = {}


def _build_nc():
    nc = bacc.Bacc()
    x_d = nc.declare_dram_parameter("x", [128, XP_COLS], F32, isOutput=False)
    w_d = nc.declare_dram_parameter("weights", [BLK, C_CORE], BF16, isOutput=False)
    b_d = nc.declare_dram_parameter("bias", [1, C_CORE], F32, isOutput=False)
    i_d = nc.declare_dram_parameter("ident", [BLK, BLK], BF16, isOutput=False)
    n_d = nc.declare_dram_parameter("ones", [1, BLK], F32, isOutput=False)
    o_d = nc.declare_dram_parameter("out", [128, XP_COLS], F32, isOutput=True)

    with tile.TileContext(nc) as tc, ExitStack() as ctx:
        consts = ctx.enter_context(tc.tile_pool(name="consts", bufs=1))
        x_pool = ctx.enter_context(tc.tile_pool(name="x", bufs=3))
        xt_pool = ctx.enter_context(tc.tile_pool(name="xt", bufs=4))
        out_pool = ctx.enter_context(tc.tile_pool(name="out", bufs=4))
        tp_pool = ctx.enter_context(tc.tile_pool(name="tp", bufs=3, space="PSUM"))
        mp_pool = ctx.enter_context(tc.tile_pool(name="mp", bufs=3, space="PSUM"))
        bp_pool = ctx.enter_context(tc.tile_pool(name="bp", bufs=1, space="PSUM"))

        ident = consts.tile([BLK, BLK], BF16)
        ones = consts.tile([1, BLK], F32)
        w_sb = consts.tile([BLK, C_CORE], BF16)
        b_row = consts.tile([1, C_CORE], F32)
        bias_sb = consts.tile([128, C_CORE], F32)

        # Consts: identity (needed by the first transpose ~10.5 us in)
        # leads the Sync ring; weights/bias lead the ACT ring ahead of
        # the x stream.
        nc.sync.dma_start(out=ident, in_=i_d[:, :])
        nc.sync.dma_start(out=ones, in_=n_d[:, :])
        nc.scalar.dma_start(out=w_sb, in_=w_d[:, :])
        nc.scalar.dma_start(out=b_row, in_=b_d[:, :])

        # Broadcast bias across partitions: [128,512] = ones.T @ b_row.
        bp = bp_pool.tile([128, C_CORE], F32)
        nc.tensor.matmul(bp, ones, b_row, start=True, stop=True)
        nc.vector.tensor_copy(bias_sb, bp)

        x_chunks = [None] * N_CHUNKS

        def emit_load(c):
            # SWDGE (gpsimd) DMA casts f32 DRAM -> bf16 SBUF inline in the
            # DMA engines: halves the SBUF-fabric bytes on the load side
            # and halves the PE transpose cost, for free.
            xc = x_pool.tile([128, G_PER_CHUNK * C_CORE], BF16)
            base = c * G_PER_CHUNK * C_CORE
            if c == 0:
                # split so the first transposes start earlier
                nc.gpsimd.dma_start(out=xc[:, 0:512], in_=x_d[:, 0:512])
                nc.gpsimd.dma_start(out=xc[:, 512:2048], in_=x_d[:, 512:2048])
                nc.gpsimd.dma_start(out=xc[:, 2048:4096], in_=x_d[:, 2048:4096])
            else:
                nc.gpsimd.dma_start(out=xc, in_=x_d[:, base:base + G_PER_CHUNK * C_CORE])
            x_chunks[c] = xc

        for c in range(2):
            emit_load(c)

        def emit_transposes(g):
            tp = tp_pool.tile([128, C_CORE], BF16)
            xc = x_chunks[g // G_PER_CHUNK]
            gb = (g % G_PER_CHUNK) * C_CORE
            for j in range(NB_CORE):
                nc.tensor.matmul(
                    tp[:, j * 128:(j + 1) * 128],
                    xc[:, gb + j * 128:gb + (j + 1) * 128],
                    ident,
                    is_transpose=True,
                    start=(j == 0),
                    stop=(j == NB_CORE - 1),
                )
            xt = xt_pool.tile([128, C_CORE], BF16)
            nc.scalar.copy(xt, tp)   # PSUM f32 -> SBUF bf16
            return xt

        xt_q = [emit_transposes(0), emit_transposes(1)]
        out_tile = None
        for g in range(GROUPS):
            if g % G_PER_OUT == 0:
                out_tile = out_pool.tile([128, G_PER_OUT * C_CORE], F32)
            # prefetch: 2 chunks (16 groups) ahead of the transposes,
            # which themselves run 2 groups ahead of the matmuls here
            if g % G_PER_CHUNK == 0 and (gc := g // G_PER_CHUNK + 2) < N_CHUNKS:
                emit_load(gc)
            xt = xt_q.pop(0)
            if g + 2 < GROUPS:
                xt_q.append(emit_transposes(g + 2))
            mp = mp_pool.tile([128, C_CORE], F32)
            for j in range(NB_CORE):
                nc.tensor.matmul(
                    mp[:, j * 128:(j + 1) * 128],
                    xt[:, j * 128:(j + 1) * 128],
                    w_sb[:, j * 128:(j + 1) * 128],
                    start=(j == 0),
                    stop=(j == NB_CORE - 1),
                )
            gi = (g % G_PER_OUT) * C_CORE
            nc.vector.tensor_add(out_tile[:, gi:gi + C_CORE], mp, bias_sb)
            # Loads ride the SWDGE queue, so BOTH HWDGE rings carry
            # stores: alternate out-tiles between them (8.4 MB each).
            if g >= GROUPS - TAIL_GROUPS:
                # tail: store per-pair alternating rings so the kernel
                # tail only waits on 512 KiB.
                if g % 2 == 1:
                    eng = nc.sync if g % 4 == 1 else nc.scalar
                    cols = slice((g - 1) * C_CORE, (g + 1) * C_CORE)
                    eng.dma_start(
                        out=o_d[:, cols],
                        in_=out_tile[:, gi - C_CORE:gi + C_CORE],
                    )
            elif g % G_PER_OUT == G_PER_OUT - 1:
                t = g // G_PER_OUT
                eng = nc.sync if t % 2 == 0 else nc.scalar
                cols = slice(t * G_PER_OUT * C_CORE, (t + 1) * G_PER_OUT * C_CORE)
                eng.dma_start(out=o_d[:, cols], in_=out_tile)

    nc.compile()
    return nc


def _get_nc():
    if "nc" not in _NC_CACHE:
        _NC_CACHE["nc"] = _build_nc()
    return _NC_CACHE["nc"]


def _run(inputs, trace=False):
    x = np.asarray(inputs["x"], dtype=np.float32)
    weights = np.asarray(inputs["weights"], dtype=np.float32)
    bias = np.asarray(inputs["bias"], dtype=np.float32)
    orig_shape = x.shape
    xf = x.reshape(B_FULL, SIZE)
    ident = np.eye(BLK, dtype=np.float32).astype(ml_dtypes.bfloat16)
    ones = np.ones((1, BLK), dtype=np.float32)

    nc = _get_nc()
    in_maps = []
    for i in range(N_CORES):
        cols = slice(i * C_CORE, (i + 1) * C_CORE)
        # pack: xp[p, g*512 + c] = xf[g*128 + p, 512*i + c]
        xp = np.ascontiguousarray(
            xf[:, cols].reshape(GROUPS, 128, C_CORE).transpose(1, 0, 2)
            .reshape(128, XP_COLS)
        )
        # weights d-major per core: [d, j*128+e] = W[4i+j, d, e], cast bf16
        w_t = np.ascontiguousarray(
            weights[i * NB_CORE:(i + 1) * NB_CORE].transpose(1, 0, 2)
            .reshape(BLK, C_CORE)
        ).astype(ml_dtypes.bfloat16)
        in_maps.append(
            {
                "x": xp,
                "weights": w_t,
                "bias": np.ascontiguousarray(bias[cols][None, :]),
                "ident": ident,
                "ones": ones,
            }
        )
    res = run_bass_kernel_spmd(
        nc, in_maps, core_ids=list(range(N_CORES)), trace=trace
    )
    out = np.empty((B_FULL, SIZE), dtype=np.float32)
    for i in range(N_CORES):
        cols = slice(i * C_CORE, (i + 1) * C_CORE)
        op = res.results[i]["out"]
        out[:, cols] = (
            op.reshape(128, GROUPS, C_CORE).transpose(1, 0, 2)
            .reshape(B_FULL, C_CORE)
        )
    return out.reshape(orig_shape), res


def kernel(**inputs):
    out, _ = _run(inputs, trace=False)
    return out
